# revision 10
# baseline (speedup 1.0000x reference)
"""Trainium2 Bass kernel for nn_Decoder (sparse_attention over genes x cells).

Strategy (per spec sharding hint): shard the n_genes axis across 8 NeuronCores;
replicate cells-side tensors. Per core (1250 genes, padded to 1280):

  phase A (on-chip): key MLP over all 8192 cells -> keyT (32, 8192);
                     query MLP over this core's genes -> queryT (32, 1280).
  phase B: for each gene-chunk (512/512/256) x cc-groups (3 cell-chunks of 128):
      scoresT psum (cells,genes) = keyT_chunk.T @ queryT_chunk      [PE, fp32r]
      logits = scoresT + gumbelT (host-transposed, packed)          [DVE, fp32]
      E = exp(logits)                                               [ACT -> fp32r]
      X_aug (101, genes) += genZ_aug_chunk.T @ E_chunk              [PE, fp32r]
        (genZ_aug has a ones column -> row 100 = softmax denominators)
      normalize: X = X_aug[:100] * (1 / X_aug[100]) via K=1 outer-product MM.

All layout transforms (gumbel transpose/packing, gen_Z transpose + ones column,
G_rep transpose, weight prescaling by 1/sqrt(32)) happen host-side in kernel().
"""
import numpy as np

import concourse.bacc as bacc
import concourse.mybir as mybir
import concourse.tile as tile
from concourse.bass_utils import run_bass_kernel_spmd

F32 = mybir.dt.float32
F32R = mybir.dt.float32r
AFT = mybir.ActivationFunctionType
ALU = mybir.AluOpType

N_GENES, N_CELLS = 10000, 8192
Z_DIM, G_REP_DIM, K_DIM, H_DIM = 100, 100, 32, 256
NCORES = 8
G_CORE = N_GENES // NCORES          # 1250
G_PAD = 1280                        # padded genes per core
CHUNKS = [(0, 512), (512, 512), (1024, 256)]   # (offset, width) gene-chunks
CC = N_CELLS // 128                 # 64 cell-chunks of 128 cells
GRP = 3                             # cell-chunks per scores/exp group
N_GROUPS = (CC + GRP - 1) // GRP    # 22 (21 full + 1 single)
INV_SQRT_DK = 1.0 / np.sqrt(np.float32(K_DIM))

_cached_nc = None


def _build_nc():
    nc = bacc.Bacc("TRN2", target_bir_lowering=False, debug=False,
                   num_devices=NCORES)

    # ---- DRAM tensors (per-core views; names = in_map keys) ----
    RAWZ = nc.dram_tensor("rawz", [Z_DIM, N_CELLS], F32R, kind="ExternalInput")
    GREPT = nc.dram_tensor("grept", [G_REP_DIM, G_PAD], F32R, kind="ExternalInput")
    GENZA = nc.dram_tensor("genza", [128, CC * 128], F32R, kind="ExternalInput")
    WZ1 = nc.dram_tensor("wz1", [Z_DIM, H_DIM], F32R, kind="ExternalInput")
    WZ2 = nc.dram_tensor("wz2", [H_DIM, K_DIM], F32R, kind="ExternalInput")
    WG1 = nc.dram_tensor("wg1", [G_REP_DIM, K_DIM], F32R, kind="ExternalInput")
    WG2S = nc.dram_tensor("wg2s", [K_DIM, K_DIM], F32R, kind="ExternalInput")
    BZ1 = nc.dram_tensor("bz1", [H_DIM, 1], F32, kind="ExternalInput")
    BZ2 = nc.dram_tensor("bz2", [K_DIM, 1], F32, kind="ExternalInput")
    BG1 = nc.dram_tensor("bg1", [K_DIM, 1], F32, kind="ExternalInput")
    BG2S = nc.dram_tensor("bg2s", [K_DIM, 1], F32, kind="ExternalInput")
    ONES = nc.dram_tensor("ones", [1, 128], F32, kind="ExternalInput")
    E100 = nc.dram_tensor("e100", [128, 1], F32, kind="ExternalInput")
    GUM = [nc.dram_tensor(f"gum{g}", [128, CC * w], F32, kind="ExternalInput")
           for g, (_, w) in enumerate(CHUNKS)]
    OUT = nc.dram_tensor("out", [Z_DIM, G_PAD], F32, kind="ExternalOutput")

    with tile.TileContext(nc) as tc:
        with (
            tc.tile_pool(name="const", bufs=1) as const,
            tc.tile_pool(name="big", bufs=2, space="PSUM") as psum_big,
            tc.tile_pool(name="acc", bufs=2, space="PSUM") as psum_acc,
            tc.tile_pool(name="work", bufs=3) as work,      # E / H1g (f32r) + T (f32)
            tc.tile_pool(name="gum", bufs=6) as gum_pool,
            tc.tile_pool(name="outp", bufs=2) as out_pool,
        ):
            # ---- load constants / weights ----
            rawz = const.tile([Z_DIM, N_CELLS], F32R)
            nc.sync.dma_start(rawz[:], RAWZ[:, :])
            grept = const.tile([G_REP_DIM, G_PAD], F32R)
            nc.sync.dma_start(grept[:], GREPT[:, :])
            genza = const.tile([128, CC * 128], F32R)
            nc.sync.dma_start(genza[:], GENZA[:, :])
            wz1 = const.tile([Z_DIM, H_DIM], F32R)
            nc.sync.dma_start(wz1[:], WZ1[:, :])
            wz2a = const.tile([128, K_DIM], F32R)
            nc.sync.dma_start(wz2a[:], WZ2[0:128, :])
            wz2b = const.tile([128, K_DIM], F32R)
            nc.sync.dma_start(wz2b[:], WZ2[128:256, :])
            wg1 = const.tile([G_REP_DIM, K_DIM], F32R)
            nc.sync.dma_start(wg1[:], WG1[:, :])
            wg2s = const.tile([K_DIM, K_DIM], F32R)
            nc.sync.dma_start(wg2s[:], WG2S[:, :])
            bz1a = const.tile([128, 1], F32)
            nc.sync.dma_start(bz1a[:], BZ1[0:128, :])
            bz1b = const.tile([128, 1], F32)
            nc.sync.dma_start(bz1b[:], BZ1[128:256, :])
            bz2 = const.tile([K_DIM, 1], F32)
            nc.sync.dma_start(bz2[:], BZ2[:, :])
            bg1 = const.tile([K_DIM, 1], F32)
            nc.sync.dma_start(bg1[:], BG1[:, :])
            bg2s = const.tile([K_DIM, 1], F32)
            nc.sync.dma_start(bg2s[:], BG2S[:, :])
            ones = const.tile([1, 128], F32)
            nc.sync.dma_start(ones[:], ONES[:, :])
            e100 = const.tile([128, 1], F32)
            nc.sync.dma_start(e100[:], E100[:, :])

            keyT = const.tile([K_DIM, N_CELLS], F32R)
            queryT = const.tile([K_DIM, G_PAD], F32R)

            # ---- phase A: query MLP (genes of this core) ----
            for off, w in CHUNKS:
                q1 = psum_big.tile([128, 512], F32, tag="ps_big")
                nc.tensor.matmul(q1[0:K_DIM, 0:w], wg1[:], grept[:, off:off + w],
                                 start=True, stop=True)
                g1g = work.tile([K_DIM, 512], F32R, tag="wk")
                nc.scalar.activation(g1g[:, 0:w], q1[0:K_DIM, 0:w], AFT.Gelu,
                                     bias=bg1[:], scale=1.0)
                q2 = psum_big.tile([128, 512], F32, tag="ps_big")
                nc.tensor.matmul(q2[0:K_DIM, 0:w], wg2s[:], g1g[:, 0:w],
                                 start=True, stop=True)
                nc.scalar.activation(queryT[:, off:off + w], q2[0:K_DIM, 0:w],
                                     AFT.Identity, bias=bg2s[:], scale=1.0)

            # ---- phase A: key MLP (all cells) ----
            for c in range(N_CELLS // 512):
                sl = slice(c * 512, (c + 1) * 512)
                h1a = psum_big.tile([128, 512], F32, tag="ps_big")
                nc.tensor.matmul(h1a[:, :], wz1[:, 0:128], rawz[:, sl],
                                 start=True, stop=True)
                h1b = psum_big.tile([128, 512], F32, tag="ps_big")
                nc.tensor.matmul(h1b[:, :], wz1[:, 128:256], rawz[:, sl],
                                 start=True, stop=True)
                h1ga = work.tile([128, 512], F32R, tag="wk")
                nc.scalar.activation(h1ga[:, :], h1a[:, :], AFT.Gelu,
                                     bias=bz1a[:], scale=1.0)
                h1gb = work.tile([128, 512], F32R, tag="wk")
                nc.scalar.activation(h1gb[:, :], h1b[:, :], AFT.Gelu,
                                     bias=bz1b[:], scale=1.0)
                kp = psum_acc.tile([128, 512], F32, tag="ps_acc")
                nc.tensor.matmul(kp[0:K_DIM, :], wz2a[:], h1ga[:, :],
                                 start=True, stop=False)
                nc.tensor.matmul(kp[0:K_DIM, :], wz2b[:], h1gb[:, :],
                                 start=False, stop=True)
                # keyT = (kp + bz2) * (1/sqrt(dk) is folded into query side)
                nc.vector.tensor_scalar(keyT[:, sl], kp[0:K_DIM, :], bz2[:], None,
                                        ALU.add)

            # ---- phase B: attention ----
            for g, (goff, w) in enumerate(CHUNKS):
                xacc = psum_acc.tile([128, 512], F32, tag="ps_acc")
                for t in range(N_GROUPS):
                    nt = min(GRP, CC - t * GRP)
                    gw = nt * w
                    gum_t = gum_pool.tile([128, GRP * 512], F32, tag="gum")
                    nc.sync.dma_start(gum_t[:, 0:gw],
                                      GUM[g][:, t * GRP * w: t * GRP * w + gw])
                    ps = psum_big.tile([128, GRP * 512], F32, tag="ps_big")
                    for j in range(nt):
                        cc = t * GRP + j
                        nc.tensor.matmul(
                            ps[:, j * 512: j * 512 + w],
                            keyT[:, cc * 128:(cc + 1) * 128],
                            queryT[:, goff:goff + w],
                            start=True, stop=True)
                    tt = work.tile([128, GRP * 512], F32, tag="wk_t")
                    et = work.tile([128, GRP * 512], F32R, tag="wk")
                    if w == 512:
                        ps_ap = ps[:, 0:gw]
                    else:
                        # psum segments are 512-strided; pack into tt/et compactly
                        ps_ap = ps[:, 0:nt * 512].rearrange(
                            "p (j x) -> p j x", j=nt)[:, :, 0:w]
                    tt_ap = (tt[:, 0:gw] if w == 512 else
                             tt[:, 0:gw].rearrange("p (j x) -> p j x", j=nt))
                    gum_ap = (gum_t[:, 0:gw] if w == 512 else
                              gum_t[:, 0:gw].rearrange("p (j x) -> p j x", j=nt))
                    nc.vector.tensor_add(tt_ap, ps_ap, gum_ap)
                    nc.scalar.activation(et[:, 0:gw], tt[:, 0:gw], AFT.Exp,
                                         bias=0.0, scale=1.0)
                    for j in range(nt):
                        cc = t * GRP + j
                        nc.tensor.matmul(
                            xacc[:, 0:w],
                            genza[:, cc * 128:(cc + 1) * 128],
                            et[:, j * w:(j + 1) * w],
                            start=(cc == 0), stop=(cc == CC - 1))
                # normalize: X = X_aug[:100] / X_aug[100]
                # (all engine reads must start at a 32-aligned partition, so
                #  extract the sums row via a selector-column matmul)
                xsb = out_pool.tile([128, 512], F32, tag="xsb")
                nc.scalar.copy(xsb[:, 0:w], xacc[:, 0:w])
                sums_ps = psum_acc.tile([128, 512], F32, tag="ps_acc")
                nc.tensor.matmul(sums_ps[0:1, 0:w], e100[:], xsb[:, 0:w],
                                 start=True, stop=True)
                rec = out_pool.tile([1, 512], F32, tag="rec")
                nc.vector.reciprocal(rec[:, 0:w], sums_ps[0:1, 0:w])
                rp = psum_acc.tile([128, 512], F32, tag="ps_acc")
                nc.tensor.matmul(rp[:, 0:w], ones[:], rec[:, 0:w],
                                 start=True, stop=True)
                rs = out_pool.tile([128, 512], F32, tag="rs")
                nc.scalar.copy(rs[:, 0:w], rp[:, 0:w])
                osb = out_pool.tile([Z_DIM, 512], F32, tag="osb")
                nc.vector.tensor_mul(osb[:, 0:w], xsb[0:Z_DIM, 0:w],
                                     rs[0:Z_DIM, 0:w])
                nc.sync.dma_start(OUT[:, goff:goff + w], osb[:, 0:w])

    nc.compile()
    return nc


def _host_prep(inputs):
    """Build the 8 per-core in_maps (all layout transforms, no model math)."""
    raw_Z = np.ascontiguousarray(inputs["raw_Z"], np.float32)
    gen_Z = np.asarray(inputs["gen_Z"], np.float32)
    G_rep = np.asarray(inputs["G_rep"], np.float32)
    gumbel = np.asarray(inputs["gumbel"], np.float32)
    s = np.float32(INV_SQRT_DK)

    # shared tensors
    gz = gen_Z.T.reshape(CC, 128, Z_DIM).transpose(1, 0, 2)   # (128, CC, 100)
    aug = np.concatenate([gz, np.ones((128, CC, 1), np.float32),
                          np.zeros((128, CC, 27), np.float32)], axis=2)
    genza = np.ascontiguousarray(aug.reshape(128, CC * 128))

    shared = {
        "rawz": raw_Z,
        "genza": genza,
        "wz1": np.ascontiguousarray(inputs["Wz1"], np.float32),
        "wz2": np.ascontiguousarray(inputs["Wz2"], np.float32),
        "wg1": np.ascontiguousarray(inputs["Wg1"], np.float32),
        "wg2s": np.ascontiguousarray(np.asarray(inputs["Wg2"], np.float32) * s),
        "bz1": np.asarray(inputs["bz1"], np.float32).reshape(H_DIM, 1),
        "bz2": np.asarray(inputs["bz2"], np.float32).reshape(K_DIM, 1),
        "bg1": np.asarray(inputs["bg1"], np.float32).reshape(K_DIM, 1),
        "bg2s": (np.asarray(inputs["bg2"], np.float32) * s).reshape(K_DIM, 1),
        "ones": np.ones((1, 128), np.float32),
        "e100": np.eye(128, 1, k=-Z_DIM, dtype=np.float32) * 1.0,
    }

    in_maps = []
    for k in range(NCORES):
        g0 = k * G_CORE
        m = dict(shared)
        grept = np.zeros((G_REP_DIM, G_PAD), np.float32)
        grept[:, :G_CORE] = G_rep[g0:g0 + G_CORE].T
        m["grept"] = grept
        gumT = np.zeros((N_CELLS, G_PAD), np.float32)
        gumT[:, :G_CORE] = gumbel[g0:g0 + G_CORE].T
        for g, (off, w) in enumerate(CHUNKS):
            blk = gumT[:, off:off + w].reshape(CC, 128, w).transpose(1, 0, 2)
            m[f"gum{g}"] = np.ascontiguousarray(blk.reshape(128, CC * w))
        in_maps.append(m)
    return in_maps


def kernel(**inputs):
    global _cached_nc
    if _cached_nc is None:
        _cached_nc = _build_nc()
    in_maps = _host_prep(inputs)
    res = run_bass_kernel_spmd(_cached_nc, in_maps, core_ids=list(range(NCORES)))
    out = np.empty((Z_DIM, N_GENES), np.float32)
    for k in range(NCORES):
        out[:, k * G_CORE:(k + 1) * G_CORE] = res.results[k]["out"][:, :G_CORE]
    return out
